# revision 1
# baseline (speedup 1.0000x reference)
"""Trainium2 Bass kernel for nn_MultiHeadAttention_3839700762945.

Full-shape contract: kernel(**inputs) takes the unsharded numpy inputs and
returns the full [4, 2048, 1024] output.

Sharding (8 cores): core c handles (batch b = c//2, head-half = c%2).
Each core computes q/k/v projections for its 8 heads (512 of the 1024 dim
columns) over the full sequence, runs attention for those heads, and emits a
partial output projection  OT_half.T @ Wo[half]  of shape [2048, 1024].
Host combines: out[b] = partial[2b] + partial[2b+1] + bo.  No collectives.

Key design points (vs the earlier staged kernel):
  - Host pre-transposes Q/K/V to [dim, seq] and pre-rounds everything to
    bf16, so the kernel does zero PE transposes and zero dtype-convert
    copies; all matmuls run at full bf16 rate.
  - Scores matmuls for a head PAIR run concurrently via PE row tiling
    (K=64 each, tile_position rows 0-63 / 64-127), halving scores PE time.
  - Softmax denominator rides as a 65th "ones" row of the AV stationary;
    normalization = DVE reciprocal + GPSIMD partition_broadcast + DVE mul.
  - Emission order is slot-scheduled: projection work (K/V/Q units) is
    interleaved between attention pair-groups so ScalarE (exp, the ~265us
    wall at 1 elem/lane/cycle) starts ~10us in and rarely starves.
"""

import sys

for _p in ("/opt/trn_rl_repo", "/opt/pypackages"):
    if _p not in sys.path:
        sys.path.insert(0, _p)

import numpy as np
import ml_dtypes

import concourse.bass as bass
import concourse.mybir as mybir
import concourse.tile as tile
import concourse.bacc as bacc

F32 = mybir.dt.float32
BF16 = mybir.dt.bfloat16
AF = mybir.ActivationFunctionType
BF = ml_dtypes.bfloat16

B, S, DIM = 4, 2048, 1024
DH = 512          # dim columns per core (8 heads x 64)
NH = 8            # heads per core
HD = 64
P = 128
NKC = DIM // P    # 8 contraction chunks for projections
NMC = DH // P     # 4 output-dim chunks
NSK = S // P      # 16 sk chunks
SQT = 512         # attention query tile
NSQT = S // SQT   # 4
EG = 2            # sk chunks per exp group
NG = NSK // EG    # 8 groups per head
NPAIR = NH // 2   # 4 head pairs
INV_SQRT_HD = 0.125
PT_BUFS = 5       # P-tile pipeline depth (pair-groups in flight)


def build_nc(reps: int = 1, mode: str = "full"):
    nc = bacc.Bacc("TRN2", target_bir_lowering=False, debug=False, num_devices=8)

    XQT = nc.dram_tensor("XQT", (DIM, S), BF16, kind="ExternalInput")
    XKT = nc.dram_tensor("XKT", (DIM, S), BF16, kind="ExternalInput")
    XVT = nc.dram_tensor("XVT", (DIM, S), BF16, kind="ExternalInput")
    WQ = nc.dram_tensor("WQ", (DIM, DH), BF16, kind="ExternalInput")
    WK = nc.dram_tensor("WK", (DIM, DH), BF16, kind="ExternalInput")
    WV = nc.dram_tensor("WV", (DIM, DH), BF16, kind="ExternalInput")
    WO = nc.dram_tensor("WO", (DH, DIM), BF16, kind="ExternalInput")
    BQ = nc.dram_tensor("BQ", (P, NMC), F32, kind="ExternalInput")
    BK = nc.dram_tensor("BK", (P, NMC), F32, kind="ExternalInput")
    BV = nc.dram_tensor("BV", (1, DH), BF16, kind="ExternalInput")
    OUT = nc.dram_tensor("OUT", (S, DIM), BF16, kind="ExternalOutput")

    with tile.TileContext(nc) as tc:
        with (
            tc.tile_pool(name="persist", bufs=1) as pc,
            tc.tile_pool(name="xstage", bufs=2) as px,
            tc.tile_pool(name="work", bufs=2) as p2,
            tc.tile_pool(name="ptile", bufs=PT_BUFS) as p4,
            tc.tile_pool(name="ps_sc", bufs=2, space="PSUM") as ps_sc,
            tc.tile_pool(name="ps_av", bufs=2, space="PSUM") as ps_av,
            tc.tile_pool(name="ps_pm", bufs=2, space="PSUM") as ps_pm,
        ):
            pools = dict(pc=pc, px=px, p2=p2, p4=p4,
                         ps_sc=ps_sc, ps_av=ps_av, ps_pm=ps_pm)
            drams = dict(XQT=XQT, XKT=XKT, XVT=XVT, WQ=WQ, WK=WK, WV=WV,
                         WO=WO, BQ=BQ, BK=BK, BV=BV, OUT=OUT)
            for _rep in range(reps):
                _emit_rep(nc, pools, drams, mode)

    nc.compile()
    return nc


def _emit_rep(nc, pools, drams, mode):
    pc, px, p2, p4 = pools["pc"], pools["px"], pools["p2"], pools["p4"]
    ps_sc, ps_av, ps_pm = pools["ps_sc"], pools["ps_av"], pools["ps_pm"]

    # ---- constants / persistent tiles -------------------------------------
    ones = pc.tile([1, SQT], BF16, tag="ones")
    nc.vector.memset(ones[:], 1.0)

    # prewarm the exp activation-table load (~1.3-2.7us) under the startup
    # DMA wait instead of in front of the first real exp
    warm = p2.tile([1, 8], F32, tag="warm")
    nc.vector.memset(warm[:], 0.0)
    warm2 = p2.tile([1, 8], BF16, tag="warm2")
    nc.scalar.activation(warm2[:], warm[:], AF.Exp)

    # DMAs ordered by first use: K-path first so scores start early.
    wsb, brow, xsb = {}, {}, {}

    def dma_w(nm, W):
        w = pc.tile([P, NKC, DH], BF16, tag=f"w{nm}", name="w")
        nc.sync.dma_start(w[:], W.ap().rearrange("(kc p) d -> p kc d", p=P))
        wsb[nm] = w

    def dma_bkq(nm, Bd):
        # per-partition bias layout [p(dh within m-chunk), m] f32 for the
        # fused tensor_scalar_add eviction
        t = pc.tile([P, NMC], F32, tag=f"b{nm}", name="t")
        nc.sync.dma_start(t[:], Bd.ap())
        brow[nm] = t

    def dma_x(nm, X):
        x = px.tile([P, NKC, S], BF16, tag=f"x{nm}", bufs=1, name="x")
        xsb[nm] = (x, X.ap().rearrange("(kc p) s -> p kc s", p=P))

    xqs = {}

    # All DMAs on the SP queue in strict priority order — the DMA fabric is
    # effectively a serial ~350GB/s resource, so global order = first-use
    # order.  First K/Q pieces are split small so the first projection
    # matmuls can start a few us in.
    xkv = drams["XKT"].ap().rearrange("(kc p) s -> p kc s", p=P)
    wkview = drams["WK"].ap().rearrange("(kc p) d -> p kc d", p=P)
    wqview = drams["WQ"].ap().rearrange("(kc p) d -> p kc d", p=P)
    xq_view = drams["XQT"].ap().rearrange("(kc p) s -> p kc s", p=P)
    xk = px.tile([P, NKC, S], BF16, tag="xk", bufs=1, name="xk")
    wk = pc.tile([P, NKC, DH], BF16, tag="wk", name="wk")
    wq = pc.tile([P, NKC, DH], BF16, tag="wq", name="wq")
    wsb["k"], wsb["q"] = wk, wq
    xsb["k"] = xk
    dma_x("v", drams["XVT"])

    def prefetch_xq(sqt):
        t = px.tile([P, NKC, SQT], BF16, tag="xq", bufs=2)
        nc.sync.dma_start(t[:], xq_view[:, :, sqt * SQT:(sqt + 1) * SQT])
        xqs[sqt] = t

    xq0 = px.tile([P, NKC, SQT], BF16, tag="xq", bufs=2, name="xq0")
    xqs[0] = xq0
    # first-use pieces: K unit m=0 half-A, then Q unit m=0 half-A, ...
    nc.sync.dma_start(xk[:, 0:4, 0:SQT], xkv[:, 0:4, 0:SQT])
    nc.sync.dma_start(wk[:, :, 0:P], wkview[:, :, 0:P])
    nc.sync.dma_start(xq0[:, 0:4, :], xq_view[:, 0:4, 0:SQT])
    nc.sync.dma_start(wq[:, :, 0:P], wqview[:, :, 0:P])
    dma_bkq("k", drams["BK"])
    dma_bkq("q", drams["BQ"])
    nc.sync.dma_start(xk[:, 4:8, 0:SQT], xkv[:, 4:8, 0:SQT])
    nc.sync.dma_start(wk[:, :, P:DH], wkview[:, :, P:DH])
    nc.sync.dma_start(xq0[:, 4:8, :], xq_view[:, 4:8, 0:SQT])
    nc.sync.dma_start(wq[:, :, P:DH], wqview[:, :, P:DH])
    xv, xvv = xsb["v"]
    nc.sync.dma_start(xv[:, :, 0:256], xvv[:, :, 0:256])
    dma_w("v", drams["WV"])
    bv_sb = pc.tile([1, DH], BF16, tag="bv")
    nc.sync.dma_start(bv_sb[:], drams["BV"].ap())
    brow["v"] = bv_sb
    nc.sync.dma_start(xv[:, :, 256:SQT], xvv[:, :, 256:SQT])
    # 256-col pieces: a consumer unblocks as soon as its columns land
    for sb in range(2, 8):
        sl = slice(sb * 256, (sb + 1) * 256)
        nc.sync.dma_start(xk[:, :, sl], xkv[:, :, sl])
        nc.sync.dma_start(xv[:, :, sl], xvv[:, :, sl])
    wo_sb = pc.tile([P, NMC, DIM], BF16, tag="wo")
    nc.sync.dma_start(wo_sb[:], drams["WO"].ap().rearrange("(kc p) d -> p kc d", p=P))
    xsb["v"] = xv

    # persistent activations
    kT = pc.tile([P, NMC, S], BF16, tag="kT")
    qT = pc.tile([P, NMC, S], BF16, tag="qT")
    vsb = pc.tile([P, NSK, NH, HD + 2], BF16, tag="vsb")
    nc.vector.memset(vsb[:, :, :, HD:HD + 1], 1.0)

    # ---- filler units (projection work interleaved into attention slots) --
    # Each unit is split into two halves (~4 matmuls each) so one filler
    # slot never delays the next scores group by much more than the PE
    # slack inside a ScalarE-paced slot.
    emitted = {"k": set(), "q": set(), "v": set()}

    def mk_kq(nm, dst, m, sb):
        """Project input `nm` chunk: dst[:, m, sb*512:(sb+1)*512]."""
        st = {}

        def kq_rhs(k):
            return (xqs[sb][:, k, :] if nm == "q"
                    else xsb[nm][:, k, sb * SQT:(sb + 1) * SQT])

        def goA():
            psp = ps_pm.tile([P, SQT], F32, tag="pm", name="psp")
            st["t"] = psp
            for k in range(4):
                nc.tensor.matmul(
                    psp[:], wsb[nm][:, k, m * P:(m + 1) * P], kq_rhs(k),
                    start=(k == 0), stop=False)

        def goB():
            psp = st["t"]
            for k in range(4, NKC):
                nc.tensor.matmul(
                    psp[:], wsb[nm][:, k, m * P:(m + 1) * P], kq_rhs(k),
                    start=False, stop=(k == NKC - 1))
            # bias fused into the eviction (per-partition scalar add)
            nc.vector.tensor_scalar_add(
                dst[:, m, sb * SQT:(sb + 1) * SQT], psp[:],
                brow[nm][:, m:m + 1])
            emitted[nm].add((m, sb))

        return [goA, goB]

    def mk_v(c, vh):
        """Project v sk-chunk c, head-half vh (heads 4vh..4vh+3) ->
        vsb[:, c, 4vh:4vh+4, 0:64].  Split by half so the deadline for a
        head-pair's AV only pulls in that half's columns."""
        st = {}
        HH = DH // 2

        def goA():
            psv = ps_pm.tile([P, HH], F32, tag="pm", name="psv")
            st["t"] = psv
            for k in range(4):
                nc.tensor.matmul(
                    psv[:], xsb["v"][:, k, c * P:(c + 1) * P],
                    wsb["v"][:, k, vh * HH:(vh + 1) * HH],
                    start=(k == 0), stop=False)

        def goB():
            psv = st["t"]
            for k in range(4, NKC):
                nc.tensor.matmul(
                    psv[:], xsb["v"][:, k, c * P:(c + 1) * P],
                    wsb["v"][:, k, vh * HH:(vh + 1) * HH],
                    start=False, stop=False)
            nc.tensor.matmul(
                psv[:], ones[0:1, 0:P], brow["v"][0:1, vh * HH:(vh + 1) * HH],
                start=False, stop=True)
            nc.vector.tensor_copy(
                vsb[:, c, 4 * vh:4 * vh + 4, 0:HD],
                psv[:].rearrange("p (h d) -> p h d", h=NH // 2))
            emitted["v"].add((c, vh))

        return [goA, goB]

    def unit_outproj(sqt, ot, m, n2, tail=False):
        """OUT[sqt*512 + m*128 .. +128, n2*512:(n2+1)*512]."""
        def go():
            pso = ps_pm.tile([P, DH], F32, tag="pm", name="pso")
            for k in range(NMC):
                nc.tensor.matmul(
                    pso[:], ot[:, k, m * P:(m + 1) * P],
                    wo_sb[:, k, n2 * DH:(n2 + 1) * DH],
                    start=(k == 0), stop=(k == NMC - 1))
            ostg = p2.tile([P, DH], BF16, tag="ostg")
            if tail:
                # ScalarE is idle after the last exp; use it for the drain
                nc.scalar.copy(ostg[:], pso[:])
            else:
                nc.vector.tensor_copy(ostg[:], pso[:])
            r0 = sqt * SQT + m * P
            nc.sync.dma_start(
                drams["OUT"].ap()[r0:r0 + P, n2 * DH:(n2 + 1) * DH], ostg[:])
        return go

    fillers = []
    # order: per pair p: K(p,0) then Q0(p) then K(p, 1..3); V half-units
    # woven — half vh is consumed by pairs 2vh/2vh+1, so its units spread
    # over the first/second half of each sq-tile's pair sweep.
    vq = [(c, 0) for c in range(NSK)] + [(c, 1) for c in range(NSK)]
    for p in range(NPAIR):
        fillers.extend(mk_kq("k", kT, p, 0))
        fillers.extend(mk_kq("q", qT, p, 0))
        for sb in range(1, 4):
            fillers.extend(mk_kq("k", kT, p, sb))
            for _ in range(2):
                if vq:
                    fillers.extend(mk_v(*vq.pop(0)))
        for _ in range(2):
            if vq:
                fillers.extend(mk_v(*vq.pop(0)))
    while vq:
        fillers.extend(mk_v(*vq.pop(0)))

    def need(nm, m, sb):
        while (m, sb) not in emitted[nm]:
            assert fillers, f"filler queue empty but need {nm} {(m, sb)}"
            fillers.pop(0)()

    def need_v(c, vh):
        while (c, vh) not in emitted["v"]:
            assert fillers, f"filler queue empty but need v {(c, vh)}"
            fillers.pop(0)()

    # prelude: first K/Q units so scores can start early
    need("k", 0, 0)
    need("q", 0, 0)

    # ---- attention --------------------------------------------------------
    def emit_scores(pair, g, sqt):
        """Returns the bf16 P tile [128, 2, EG, SQT] for this pair-group."""
        sq0 = sqt * SQT
        need("k", pair, (g * EG) // 4)
        need("k", pair, (g * EG + EG - 1) // 4)
        # prefetch margin: pull the next groups' kT dependency two slots early
        gn = min(g + 2, NG - 1)
        need("k", pair, (gn * EG + EG - 1) // 4)
        need("q", pair, sqt)
        ptt = p4.tile([P, 2, EG, SQT], BF16, tag="pt")
        for half in range(2):
            pss = ps_sc.tile([P, EG, SQT], F32, tag="sc")
            b0 = half * HD
            for ci in range(EG):
                c = g * EG + ci
                nc.tensor.matmul(
                    pss[:, ci, :],
                    kT[b0:b0 + HD, pair, c * P:(c + 1) * P],
                    qT[b0:b0 + HD, pair, sq0:sq0 + SQT],
                    start=True, stop=True,
                    tile_position=(b0, 0))
            nc.scalar.activation(ptt[:, half, :, :], pss[:], AF.Exp,
                                 scale=INV_SQRT_HD)
        return ptt

    def emit_av(pair, g, ptt, psavs):
        for half in range(2):
            h = pair * 2 + half
            if g == 0:
                psavs[half] = ps_av.tile([P, SQT], F32, tag="av", name="psav")
            for ci in range(EG):
                c = g * EG + ci
                need_v(c, pair // 2)
                nc.tensor.matmul(
                    psavs[half][0:HD + 1, :],
                    vsb[:, c, h, 0:HD + 1],
                    ptt[:, half, ci, :],
                    start=(c == 0), stop=(c == NSK - 1))

    def emit_norm(pair, ot, psavs):
        for half in range(2):
            h = pair * 2 + half
            psav = psavs[half]
            # reciprocal_approx_fast reads garbage from a PSUM source; stage
            # the row-sum into SBUF first.
            rsh = p2.tile([1, SQT], F32, tag="rsh")
            rst = p2.tile([1, SQT], F32, tag="rst")
            nc.vector.tensor_copy(rst[:], psav[HD:HD + 1, :])
            nc.vector.reciprocal_approx_fast(rsh[:], rst[:])
            bcs = p2.tile([HD, SQT], F32, tag="bcs")
            nc.gpsimd.partition_broadcast(bcs[:], rsh[:], channels=HD)
            base = half * HD
            mch = pair
            nc.vector.tensor_mul(ot[base:base + HD, mch, :],
                                 psav[0:HD, :], bcs[:])

    # software pipeline over (sqt, pair, group); AV lags scores by 1 group.
    ot_prev = None
    sqt_prev = None
    slot = 0
    pending = [None]
    for sqt in range(NSQT):
        if sqt + 1 < NSQT:
            prefetch_xq(sqt + 1)
        ot = p2.tile([P, NMC, SQT], BF16, tag="ot")
        for pair in range(NPAIR):
            psavs = {}
            ptts = {}
            for g in range(NG + 1):
                if g < NG:
                    ptts[g] = emit_scores(pair, g, sqt)
                    if g == 0 and pending[0] is not None:
                        # previous pair's epilogue runs behind this pair's
                        # first scores so ScalarE isn't starved at the
                        # pair boundary
                        pending[0]()
                        pending[0] = None
                    # one filler per group keeps PE fed while ACT drains;
                    # more in the first slots while the exp pipeline fills
                    # (PE must not sit on the scores-PSUM wall with fillers
                    # trapped behind it in the FIFO)
                    nfill = 3 if slot < 4 else (2 if slot < 12 else 1)
                    for _ in range(nfill):
                        if fillers:
                            fillers.pop(0)()
                    slot += 1
                if g >= 1:
                    emit_av(pair, g - 1, ptts.pop(g - 1), psavs)

            def epi(pair=pair, ot=ot, psavs=psavs, ot_prev=ot_prev,
                    sqt_prev=sqt_prev, sqt=sqt):
                emit_norm(pair, ot, psavs)
                # spread previous sq-tile's output projection: pair p
                # emits tiles (m=p, n2=0..1) -> 8 tiles per sq-tile
                if ot_prev is not None:
                    for n2 in range(2):
                        unit_outproj(sqt_prev, ot_prev, pair, n2)()
                # next sq-tile's Q projection, one m-chunk per pair
                if sqt + 1 < NSQT:
                    for half_fn in mk_kq("q", qT, pair, sqt + 1):
                        half_fn()

            if pair < NPAIR - 1:
                pending[0] = epi
            else:
                # last pair of the sq-tile: run now so no deferred reader
                # of an old ot buffer crosses the sq-tile boundary
                epi()
        ot_prev, sqt_prev = ot, sqt
    # drain remaining fillers and the last out-projection; alternate the
    # evictions between ScalarE and DVE so the drain chains run in parallel
    while fillers:
        fillers.pop(0)()
    for i, (m, n2) in enumerate([(m, n2) for m in range(NMC) for n2 in range(2)]):
        unit_outproj(sqt_prev, ot_prev, m, n2, tail=(i % 2 == 0))()


_cached = {}


def _get_nc(reps: int = 1, mode: str = "full"):
    key = (reps, mode)
    if key not in _cached:
        _cached[key] = build_nc(reps, mode)
    return _cached[key]


def _bf(x):
    return np.ascontiguousarray(np.asarray(x, np.float32).astype(BF))


def make_in_maps(Q, K, V, Wq, bq, Wk, bk, Wv, bv, Wo, bo):
    xqt = [_bf(np.asarray(Q[b], np.float32).T) for b in range(B)]
    xkt = [_bf(np.asarray(K[b], np.float32).T) for b in range(B)]
    xvt = [_bf(np.asarray(V[b], np.float32).T) for b in range(B)]
    halves = []
    for half in range(2):
        sl = slice(half * DH, (half + 1) * DH)
        halves.append({
            "WQ": _bf(np.asarray(Wq)[:, sl]),
            "WK": _bf(np.asarray(Wk)[:, sl]),
            "WV": _bf(np.asarray(Wv)[:, sl]),
            "WO": _bf(np.asarray(Wo)[sl, :]),
            "BQ": np.ascontiguousarray(
                np.asarray(bq, np.float32)[sl].reshape(NMC, P).T),
            "BK": np.ascontiguousarray(
                np.asarray(bk, np.float32)[sl].reshape(NMC, P).T),
            "BV": _bf(np.asarray(bv)[sl]).reshape(1, DH),
        })
    in_maps = []
    for c in range(8):
        b, half = divmod(c, 2)
        m = {"XQT": xqt[b], "XKT": xkt[b], "XVT": xvt[b]}
        m.update(halves[half])
        in_maps.append(m)
    return in_maps


def combine(results, bo):
    bo = np.asarray(bo, dtype=np.float32)
    return np.stack([
        np.asarray(results[2 * b]["OUT"], np.float32)
        + np.asarray(results[2 * b + 1]["OUT"], np.float32) + bo
        for b in range(B)
    ])


def kernel(Q, K, V, Wq, bq, Wk, bk, Wv, bv, Wo, bo):
    from concourse.bass_utils import run_bass_kernel_spmd
    nc = _get_nc(1)
    in_maps = make_in_maps(Q, K, V, Wq, bq, Wk, bk, Wv, bv, Wo, bo)
    res = run_bass_kernel_spmd(nc, in_maps, core_ids=list(range(8)))
    return combine(res.results, bo)



# revision 56
# speedup vs baseline: 1.1815x; 1.1815x over previous
"""Trainium2 Bass kernel for nn_MultiHeadAttention_3839700762945.

Full-shape contract: kernel(**inputs) takes the unsharded numpy inputs and
returns the full [4, 2048, 1024] output.

Sharding (8 cores): core c handles (batch b = c//2, head-half = c%2).
Each core computes q/k/v projections for its 8 heads (512 of the 1024 dim
columns) over the full sequence, runs attention for those heads, and emits a
partial output projection  OT_half.T @ Wo[half]  of shape [2048, 1024].
Host combines: out[b] = partial[2b] + partial[2b+1] + bo.  No collectives.

Key design points (vs the earlier staged kernel):
  - Host pre-transposes Q/K/V to [dim, seq] and pre-rounds everything to
    bf16, so the kernel does zero PE transposes and zero dtype-convert
    copies; all matmuls run at full bf16 rate.
  - Scores matmuls for a head PAIR run concurrently via PE row tiling
    (K=64 each, tile_position rows 0-63 / 64-127), halving scores PE time.
  - Softmax denominator rides as a 65th "ones" row of the AV stationary;
    normalization = DVE reciprocal + GPSIMD partition_broadcast + DVE mul.
  - Emission order is slot-scheduled: projection work (K/V/Q units) is
    interleaved between attention pair-groups so ScalarE (exp, the ~265us
    wall at 1 elem/lane/cycle) starts ~10us in and rarely starves.
"""

import sys

for _p in ("/opt/trn_rl_repo", "/opt/pypackages"):
    if _p not in sys.path:
        sys.path.insert(0, _p)

import numpy as np
import ml_dtypes

import concourse.bass as bass
import concourse.mybir as mybir
import concourse.tile as tile
import concourse.bacc as bacc

F32 = mybir.dt.float32
BF16 = mybir.dt.bfloat16
F8 = mybir.dt.float8e4
AF = mybir.ActivationFunctionType
DR = mybir.MatmulPerfMode.DoubleRow
BF = ml_dtypes.bfloat16
NF8 = ml_dtypes.float8_e4m3

B, S, DIM = 4, 2048, 1024
DH = 512          # dim columns per core (8 heads x 64)
NH = 8            # heads per core
HD = 64
P = 128
NKC = DIM // P    # 8 contraction chunks for projections
NKP = NKC // 2    # 4 DoubleRow contraction pairs
NMC = DH // P     # 4 output-dim chunks
NSK = S // P      # 16 sk chunks
SQT = 512         # attention query tile
NSQT = S // SQT   # 4
EG = 2            # sk chunks per exp group
NG = NSK // EG    # 8 groups per head
NPAIR = NH // 2   # 4 head pairs
INV_SQRT_HD = 0.125
FLOOR_SCALE = 0.0  # scheduler model-time floors; 0 disables
# Schraudolph fast-exp on DVE: bf16 bits = rint(128*(s*0.125*log2e + 127 - C))
# (DVE fp32->int16 conversion is round-to-nearest, HW-verified); written
# through an int16 bitcast view of the bf16 P tile.  Max rel err ~3%
# (sawtooth).  Offload is split by HALF: half0's exps always on ScalarE,
# half1's always on DVE (sq-tiles >= 1; sqt0's DVE is loaded with
# projection evictions).  Each ps_sc buffer position is then consumed by
# one fixed engine, so the two exp chains run decoupled in parallel.
SCH_C1 = 128.0 * 0.125 * 1.4426950408889634
SCH_C2 = 128.0 * (127.0 - 0.0579848)
PT_BUFS = 5       # P-tile pipeline depth (pair-groups in flight)
WSCALE = 16.0     # host multiplies W by this before fp8 cast (keeps W normal)
OTSCALE = 8.0     # attn-out scaled by this before fp8 ot (keeps ot normal)
EXP_BIAS = -5.0   # exp(s-5): scores reach ~8.7, fp8e4m3 infs past 240, so
                  # keep P <= ~e^5.5 with margin; cancels in the softmax
                  # normalization (weights below ~1e-4 flush to 0, harmless)
VPAD = 72         # vsb row padding (DoubleRow needs 16B-aligned Ko stride)


def build_nc(reps: int = 1, mode: str = "full"):
    nc = bacc.Bacc("TRN2", target_bir_lowering=False, debug=False, num_devices=8)

    XQT = nc.dram_tensor("XQT", (DIM, S), BF16, kind="ExternalInput")
    XKT = nc.dram_tensor("XKT", (DIM, S), BF16, kind="ExternalInput")
    XVT = nc.dram_tensor("XVT", (DIM, S), BF16, kind="ExternalInput")
    WQ = nc.dram_tensor("WQ", (DIM, DH), BF16, kind="ExternalInput")
    WK = nc.dram_tensor("WK", (DIM, DH), BF16, kind="ExternalInput")
    WV = nc.dram_tensor("WV", (DIM, DH), BF16, kind="ExternalInput")
    WO = nc.dram_tensor("WO", (DH, DIM), BF16, kind="ExternalInput")
    BQ = nc.dram_tensor("BQ", (P, NMC), F32, kind="ExternalInput")
    BK = nc.dram_tensor("BK", (P, NMC), F32, kind="ExternalInput")
    BV = nc.dram_tensor("BV", (1, DH), BF16, kind="ExternalInput")
    OUT = nc.dram_tensor("OUT", (S, DIM), BF16, kind="ExternalOutput")

    with tile.TileContext(nc) as tc:
        with (
            tc.tile_pool(name="persist", bufs=1) as pc,
            tc.tile_pool(name="xstage", bufs=2) as px,
            tc.tile_pool(name="work", bufs=2) as p2,
            tc.tile_pool(name="ptile", bufs=PT_BUFS) as p4,
            tc.tile_pool(name="ps_sc", bufs=2, space="PSUM") as ps_sc,
            tc.tile_pool(name="ps_av", bufs=2, space="PSUM") as ps_av,
            tc.tile_pool(name="ps_pm", bufs=2, space="PSUM") as ps_pm,
        ):
            pools = dict(pc=pc, px=px, p2=p2, p4=p4,
                         ps_sc=ps_sc, ps_av=ps_av, ps_pm=ps_pm)
            drams = dict(XQT=XQT, XKT=XKT, XVT=XVT, WQ=WQ, WK=WK, WV=WV,
                         WO=WO, BQ=BQ, BK=BK, BV=BV, OUT=OUT)
            for _rep in range(reps):
                _emit_rep(nc, tc, pools, drams, mode)

    nc.compile()
    return nc


def _emit_rep(nc, tc, pools, drams, mode):
    pc, px, p2, p4 = pools["pc"], pools["px"], pools["p2"], pools["p4"]
    ps_sc, ps_av, ps_pm = pools["ps_sc"], pools["ps_av"], pools["ps_pm"]

    # ---- constants / persistent tiles -------------------------------------
    ones = pc.tile([1, SQT], BF16, tag="ones")
    nc.vector.memset(ones[:], 1.0)

    # prewarm the exp activation-table load (~1.3-2.7us) under the startup
    # DMA wait instead of in front of the first real exp
    warm = p2.tile([1, 8], F32, tag="warm")
    nc.vector.memset(warm[:], 0.0)
    warm2 = p2.tile([1, 8], BF16, tag="warm2")
    nc.scalar.activation(warm2[:], warm[:], AF.Exp)

    # DMAs ordered by first use: K-path first so scores start early.
    wsb, brow, xsb = {}, {}, {}

    def dma_w(nm, W):
        w = pc.tile([P, NKC, DH], BF16, tag=f"w{nm}", name="w")
        nc.sync.dma_start(w[:], W.ap().rearrange("(kc p) d -> p kc d", p=P))
        wsb[nm] = w

    def dma_bkq(nm, Bd):
        # per-partition bias layout [p(dh within m-chunk), m] f32 for the
        # fused tensor_scalar_add eviction
        t = pc.tile([P, NMC], F32, tag=f"b{nm}", name="t")
        nc.sync.dma_start(t[:], Bd.ap())
        brow[nm] = t

    def dma_x(nm, X):
        x = px.tile([P, NKC, S], BF16, tag=f"x{nm}", bufs=1, name="x")
        xsb[nm] = (x, X.ap().rearrange("(kc p) s -> p kc s", p=P))

    xqs = {}

    # All DMAs on the SP queue in strict priority order — the DMA fabric is
    # effectively a serial ~350GB/s resource, so global order = first-use
    # order.  First K/Q pieces are split small so the first projection
    # matmuls can start a few us in.
    xkv = drams["XKT"].ap().rearrange("(kc p) s -> p kc s", p=P)
    wkview = drams["WK"].ap().rearrange("(kc p) d -> p kc d", p=P)
    wqview = drams["WQ"].ap().rearrange("(kc p) d -> p kc d", p=P)
    xq_view = drams["XQT"].ap().rearrange("(kc p) s -> p kc s", p=P)
    xk = px.tile([P, NKC, S], BF16, tag="xk", bufs=1, name="xk")
    wk = pc.tile([P, NKC, DH], BF16, tag="wk", name="wk")
    wq = pc.tile([P, NKC, DH], BF16, tag="wq", name="wq")
    wsb["k"], wsb["q"] = wk, wq
    xsb["k"] = xk
    dma_x("v", drams["XVT"])

    def prefetch_xq(sqt):
        t = px.tile([P, NKC, SQT], BF16, tag="xq", bufs=2)
        nc.sync.dma_start(t[:], xq_view[:, :, sqt * SQT:(sqt + 1) * SQT])
        xqs[sqt] = t

    xq0 = px.tile([P, NKC, SQT], BF16, tag="xq", bufs=2, name="xq0")
    xqs[0] = xq0
    # first-use pieces: K unit m=0 half-A, then Q unit m=0 half-A, ...
    nc.sync.dma_start(xk[:, 0:4, 0:SQT], xkv[:, 0:4, 0:SQT])
    nc.sync.dma_start(wk[:, :, 0:P], wkview[:, :, 0:P])
    nc.sync.dma_start(xq0[:, 0:4, :], xq_view[:, 0:4, 0:SQT])
    nc.sync.dma_start(wq[:, :, 0:P], wqview[:, :, 0:P])
    dma_bkq("k", drams["BK"])
    dma_bkq("q", drams["BQ"])
    nc.sync.dma_start(xk[:, 4:8, 0:SQT], xkv[:, 4:8, 0:SQT])
    nc.sync.dma_start(wk[:, :, P:DH], wkview[:, :, P:DH])
    nc.sync.dma_start(xq0[:, 4:8, :], xq_view[:, 4:8, 0:SQT])
    nc.sync.dma_start(wq[:, :, P:DH], wqview[:, :, P:DH])
    # ALL remaining xk pieces first: the exp chain of sqt0-pair0 is paced
    # by kT availability (scores deadline), while the AV consumers of xv
    # ride several groups behind through the P-tile pipeline depth
    for sb in range(2, 8):
        sl = slice(sb * 256, (sb + 1) * 256)
        nc.sync.dma_start(xk[:, :, sl], xkv[:, :, sl])
    xv, xvv = xsb["v"]
    nc.sync.dma_start(xv[:, :, 0:256], xvv[:, :, 0:256])
    dma_w("v", drams["WV"])
    bv_sb = pc.tile([1, DH], BF16, tag="bv")
    nc.sync.dma_start(bv_sb[:], drams["BV"].ap())
    brow["v"] = bv_sb
    nc.sync.dma_start(xv[:, :, 256:SQT], xvv[:, :, 256:SQT])
    for sb in range(2, 8):
        sl = slice(sb * 256, (sb + 1) * 256)
        nc.sync.dma_start(xv[:, :, sl], xvv[:, :, sl])
    # wo first used ~100us in (outproj of sqt0 runs during sqt1)
    wo_sb = pc.tile([P, NMC, DIM], BF16, tag="wo")
    nc.sync.dma_start(wo_sb[:], drams["WO"].ap().rearrange("(kc p) d -> p kc d", p=P))
    xsb["v"] = xv

    # estimated DMA landing times in us (model hints for the scheduler):
    # the SP queue drains serially at ~300GB/s after a ~7us NRT preamble.
    # t_kunit[sb] / t_vchunk[c] mark when K unit sb / V chunk c inputs
    # land; used as tile_wait_until floors so the static PE order never
    # places a DMA-blocked filler ahead of ready attention work.
    R = 3.33 * FLOOR_SCALE  # us per MB
    t = 7.0 + (2.0 + 0.26) * R            # xk p1 + wk half1
    t_kunit = {0: 0.0}
    t += (1.0 + 0.26) * R                 # xq p1, wq1
    t += (2.0 + 0.79) * R                 # xk p2, wk2
    t_kunit[0] = t
    t += (1.0 + 0.79) * R                 # xq p2, wq2
    t_xq0 = t
    t += (0.5 + 1.0 + 0.01 + 0.5) * R     # xv p1, wv, bv, xv p2
    t_xvp = {0: t, 1: t}
    for _p in range(2, 8):
        t += 0.5 * R
        t_kunit[_p // 2] = t              # xk piece _p (keys _p*256..)
        t += 0.5 * R
        t_xvp[_p] = t
    t_wo = t + 1.0 * R
    t_vchunk = {c: t_xvp[c // 2] for c in range(NSK)}

    # persistent activations.  vsb is laid out for DoubleRow AV: group g's
    # two sk-chunks are the Ko=2 dim; VPAD keeps the Ko stride 16B-aligned.
    kT = pc.tile([P, NMC, S], BF16, tag="kT")
    qT = pc.tile([P, NMC, S], BF16, tag="qT")
    vsb = pc.tile([P, NSK, NH, HD + 2], BF16, tag="vsb")
    nc.vector.memset(vsb[:, :, :, HD:HD + 1], 1.0)

    # ---- filler units (projection work interleaved into attention slots) --
    # Each unit is split into two halves (~4 matmuls each) so one filler
    # slot never delays the next scores group by much more than the PE
    # slack inside a ScalarE-paced slot.
    emitted = {"k": set(), "q": set(), "v": set()}

    def mk_kq(nm, dst, m, sb):
        """Project input `nm` chunk: dst[:, m, sb*512:(sb+1)*512]."""
        st = {}
        t_rdy = (t_xq0 if (nm == "q" and sb == 0) else
                 t_kunit[sb] if nm == "k" else 0.0)

        def kq_rhs(k):
            return (xqs[sb][:, k, :] if nm == "q"
                    else xsb[nm][:, k, sb * SQT:(sb + 1) * SQT])

        def goA():
            with tc.tile_wait_until(t_rdy / 1000.0, enable=FLOOR_SCALE > 0 and t_rdy > 0):
                psp = ps_pm.tile([P, SQT], F32, tag="pm", name="psp")
                st["t"] = psp
                for k in range(4):
                    nc.tensor.matmul(
                        psp[:], wsb[nm][:, k, m * P:(m + 1) * P], kq_rhs(k),
                        start=(k == 0), stop=False)

        def goB():
            with tc.tile_wait_until(t_rdy / 1000.0, enable=FLOOR_SCALE > 0 and t_rdy > 0):
                psp = st["t"]
                for k in range(4, NKC):
                    nc.tensor.matmul(
                        psp[:], wsb[nm][:, k, m * P:(m + 1) * P], kq_rhs(k),
                        start=False, stop=(k == NKC - 1))
                # bias fused into the eviction (per-partition scalar add)
                nc.vector.tensor_scalar_add(
                    dst[:, m, sb * SQT:(sb + 1) * SQT], psp[:],
                    brow[nm][:, m:m + 1])
            emitted[nm].add((m, sb))

        return [goA, goB]

    def mk_v(c):
        """Project v sk-chunk c (all 8 heads, full DH=512 moving width) ->
        vsb[:, c, :, 0:64]."""
        st = {}
        t_rdy = t_vchunk[c]

        def goA():
            with tc.tile_wait_until(t_rdy / 1000.0, enable=FLOOR_SCALE > 0):
                psv = ps_pm.tile([P, DH], F32, tag="pm", name="psv")
                st["t"] = psv
                for k in range(4):
                    nc.tensor.matmul(
                        psv[:], xsb["v"][:, k, c * P:(c + 1) * P],
                        wsb["v"][:, k, :],
                        start=(k == 0), stop=False)

        def goB():
            with tc.tile_wait_until(t_rdy / 1000.0, enable=FLOOR_SCALE > 0):
                psv = st["t"]
                for k in range(4, NKC):
                    nc.tensor.matmul(
                        psv[:], xsb["v"][:, k, c * P:(c + 1) * P],
                        wsb["v"][:, k, :],
                        start=False, stop=False)
                nc.tensor.matmul(
                    psv[:], ones[0:1, 0:P], brow["v"][0:1, :],
                    start=False, stop=True)
                nc.vector.tensor_copy(
                    vsb[:, c, :, 0:HD],
                    psv[:].rearrange("p (h d) -> p h d", h=NH))
            emitted["v"].add(c)

        return [goA, goB]

    def unit_outproj(sqt, ot, m, n2, tail=False):
        """OUT[sqt*512 + m*128 .. +128, n2*512:(n2+1)*512]."""
        def go():
            with tc.tile_wait_until(t_wo / 1000.0, enable=FLOOR_SCALE > 0):
                pso = ps_pm.tile([P, DH], F32, tag="pm", name="pso")
                for k in range(NMC):
                    nc.tensor.matmul(
                        pso[:], ot[:, k, m * P:(m + 1) * P],
                        wo_sb[:, k, n2 * DH:(n2 + 1) * DH],
                        start=(k == 0), stop=(k == NMC - 1))
                ostg = p2.tile([P, DH], BF16, tag="ostg")
                if tail:
                    # ScalarE is idle after the last exp; use it for the
                    # drain
                    nc.scalar.copy(ostg[:], pso[:])
                else:
                    nc.vector.tensor_copy(ostg[:], pso[:])
                r0 = sqt * SQT + m * P
                nc.sync.dma_start(
                    drams["OUT"].ap()[r0:r0 + P, n2 * DH:(n2 + 1) * DH],
                    ostg[:])
        return go

    fillers = []
    # emission order = joint (DMA arrival, deadline) order: pool rotation
    # follows emission, so a unit emitted before its DMA lands would trap
    # later-emitted ready units behind its PSUM buffer.
    # deadline order: pair0's K/Q first, then all V chunks (every chunk is
    # consumed by pair0's AV sweep), then later pairs' K/Q.  The Tile
    # scheduler treats emission order as priority and dispatches by actual
    # operand readiness.
    fillers.extend(mk_kq("k", kT, 0, 0))
    fillers.extend(mk_kq("q", qT, 0, 0))
    for sb in range(1, 4):
        fillers.extend(mk_kq("k", kT, 0, sb))
    for c in range(NSK):
        fillers.extend(mk_v(c))
    for p in range(1, NPAIR):
        fillers.extend(mk_kq("k", kT, p, 0))
        fillers.extend(mk_kq("q", qT, p, 0))
        for sb in range(1, 4):
            fillers.extend(mk_kq("k", kT, p, sb))

    def need(nm, m, sb):
        while (m, sb) not in emitted[nm]:
            assert fillers, f"filler queue empty but need {nm} {(m, sb)}"
            fillers.pop(0)()

    def need_v(c):
        while c not in emitted["v"]:
            assert fillers, f"filler queue empty but need v {c}"
            fillers.pop(0)()

    # prelude: first K/Q units so scores can start early
    need("k", 0, 0)
    need("q", 0, 0)

    # ---- attention --------------------------------------------------------
    def emit_scores(pair, g, sqt, margin):
        """Returns the bf16 P tile [128, 2, EG, SQT] for this pair-group."""
        sq0 = sqt * SQT
        need("k", pair, (g * EG) // 4)
        need("k", pair, (g * EG + EG - 1) // 4)
        if margin:
            # prefetch margin: pull later groups' kT dependency early
            gn = min(g + margin, NG - 1)
            need("k", pair, (gn * EG + EG - 1) // 4)
        need("q", pair, sqt)
        ptt = p4.tile([P, 2, EG, SQT], BF16, tag="pt")
        for half in range(2):
            pss = ps_sc.tile([P, EG, SQT], F32, tag="sc")
            b0 = half * HD
            for ci in range(EG):
                c = g * EG + ci
                nc.tensor.matmul(
                    pss[:, ci, :],
                    kT[b0:b0 + HD, pair, c * P:(c + 1) * P],
                    qT[b0:b0 + HD, pair, sq0:sq0 + SQT],
                    start=True, stop=True,
                    tile_position=(b0, 0))
            nc.scalar.activation(ptt[:, half, :, :], pss[:], AF.Exp,
                                 scale=INV_SQRT_HD)
        return ptt

    def emit_av(pair, g, ptt, psavs):
        need_v(g * EG)
        need_v(g * EG + 1)
        for half in range(2):
            h = pair * 2 + half
            if g == 0:
                psavs[half] = ps_av.tile([P, SQT], F32, tag="av", name="psav")
            for ci in range(EG):
                c = g * EG + ci
                nc.tensor.matmul(
                    psavs[half][0:HD + 1, :],
                    vsb[:, c, h, 0:HD + 1],
                    ptt[:, half, ci, :],
                    start=(c == 0), stop=(c == NSK - 1))

    def emit_norm(pair, ot, psavs):
        for half in range(2):
            h = pair * 2 + half
            psav = psavs[half]
            # reciprocal_approx_fast reads garbage from a PSUM source; stage
            # the row-sum into SBUF first.
            rsh = p2.tile([1, SQT], F32, tag="rsh")
            rst = p2.tile([1, SQT], F32, tag="rst")
            nc.vector.tensor_copy(rst[:], psav[HD:HD + 1, :])
            nc.vector.reciprocal_approx_fast(rsh[:], rst[:])
            bcs = p2.tile([HD, SQT], F32, tag="bcs")
            nc.gpsimd.partition_broadcast(bcs[:], rsh[:], channels=HD)
            base = half * HD
            mch = pair
            nc.vector.tensor_mul(ot[base:base + HD, mch, :],
                                 psav[0:HD, :], bcs[:])

    # ---- slot-scheduled software pipeline over (sqt, pair, group) ---------
    # Every slot emits, in FIFO order: [scores(g)] [one epilogue slice]
    # [fillers] [AV(g-1)].  Scores go FIRST so ScalarE (exp, the pacing
    # engine) is unblocked at the earliest possible point; AV goes LAST
    # because it waits on exp(g-1) anyway and must not head-block the
    # slice/filler work in the strict-FIFO PE queue.  Pair epilogues
    # (normalize, prev-sqt outproj, next-sqt qproj) are sliced and paced
    # one per slot instead of running as a monolithic block that would
    # starve ScalarE at pair boundaries.
    ot_prev = None
    sqt_prev = None
    ot = None
    prev_ctx = None   # (pair, psavs, ptt7, ot, sqt, ot_prev, sqt_prev)
    slices = []

    def push_boundary_slices(ctx):
        p_pair, p_psavs, _p_ptt, p_ot, p_sqt, p_ot_prev, p_sqt_prev = ctx
        slices.append(lambda: emit_norm(p_pair, p_ot, p_psavs))
        if p_ot_prev is not None:
            for n2 in range(2):
                slices.append(unit_outproj(p_sqt_prev, p_ot_prev, p_pair, n2))
        if p_sqt + 1 < NSQT:
            for half_fn in mk_kq("q", qT, p_pair, p_sqt + 1):
                slices.append(half_fn)

    for sqt in range(NSQT):
        if sqt + 1 < NSQT:
            prefetch_xq(sqt + 1)
        ot = p2.tile([P, NMC, SQT], BF16, tag="ot")
        for pair in range(NPAIR):
            psavs = {}
            ptts = {}
            for g in range(NG):
                margin = 0 if (sqt == 0 and pair == 0) else 2
                ptts[g] = emit_scores(pair, g, sqt, margin)
                boundary = (g == 0 and prev_ctx is not None)
                if not boundary and slices:
                    slices.pop(0)()
                nfill = 2 if sqt == 0 else 1
                for _ in range(nfill):
                    if fillers:
                        fillers.pop(0)()
                if boundary:
                    # previous pair's lagged last AV group goes LAST in the
                    # slot (it waits on the previous pair's final exp); its
                    # epilogue slices queue behind it (norm first: frees the
                    # psav tiles before this pair's AV allocates at g==1)
                    p_pair, p_psavs, p_ptt = prev_ctx[0], prev_ctx[1], prev_ctx[2]
                    emit_av(p_pair, NG - 1, p_ptt, p_psavs)
                    push_boundary_slices(prev_ctx)
                    prev_ctx = None
                elif g >= 1:
                    emit_av(pair, g - 1, ptts.pop(g - 1), psavs)
            prev_ctx = (pair, psavs, ptts.pop(NG - 1), ot, sqt,
                        ot_prev, sqt_prev)
        ot_prev, sqt_prev = ot, sqt

    # drain: last pair's AV + epilogue, remaining slices/fillers, and the
    # final sq-tile's out-projection; alternate evictions between ScalarE
    # (idle after the last exp) and DVE so the drain chains run in parallel
    p_pair, p_psavs, p_ptt = prev_ctx[0], prev_ctx[1], prev_ctx[2]
    emit_av(p_pair, NG - 1, p_ptt, p_psavs)
    push_boundary_slices(prev_ctx)
    for s in slices:
        s()
    while fillers:
        fillers.pop(0)()
    for i, (m, n2) in enumerate([(m, n2) for m in range(NMC) for n2 in range(2)]):
        unit_outproj(sqt_prev, ot_prev, m, n2, tail=(i % 2 == 0))()


_cached = {}


def _get_nc(reps: int = 1, mode: str = "full"):
    key = (reps, mode)
    if key not in _cached:
        _cached[key] = build_nc(reps, mode)
    return _cached[key]


def _bf(x):
    return np.ascontiguousarray(np.asarray(x, np.float32).astype(BF))


def _f8(x):
    return np.ascontiguousarray(np.asarray(x, np.float32).astype(NF8))


def make_in_maps(Q, K, V, Wq, bq, Wk, bk, Wv, bv, Wo, bo):
    xqt = [_bf(np.asarray(Q[b], np.float32).T) for b in range(B)]
    xkt = [_bf(np.asarray(K[b], np.float32).T) for b in range(B)]
    xvt = [_bf(np.asarray(V[b], np.float32).T) for b in range(B)]
    halves = []
    for half in range(2):
        sl = slice(half * DH, (half + 1) * DH)
        halves.append({
            "WQ": _bf(np.asarray(Wq)[:, sl]),
            "WK": _bf(np.asarray(Wk)[:, sl]),
            "WV": _bf(np.asarray(Wv)[:, sl]),
            "WO": _bf(np.asarray(Wo)[sl, :]),
            "BQ": np.ascontiguousarray(
                np.asarray(bq, np.float32)[sl].reshape(NMC, P).T),
            "BK": np.ascontiguousarray(
                np.asarray(bk, np.float32)[sl].reshape(NMC, P).T),
            "BV": _bf(np.asarray(bv)[sl]).reshape(1, DH),
        })
    in_maps = []
    for c in range(8):
        b, half = divmod(c, 2)
        m = {"XQT": xqt[b], "XKT": xkt[b], "XVT": xvt[b]}
        m.update(halves[half])
        in_maps.append(m)
    return in_maps


def combine(results, bo):
    bo = np.asarray(bo, dtype=np.float32)
    return np.stack([
        np.asarray(results[2 * b]["OUT"], np.float32)
        + np.asarray(results[2 * b + 1]["OUT"], np.float32) + bo
        for b in range(B)
    ])


def kernel(Q, K, V, Wq, bq, Wk, bk, Wv, bv, Wo, bo):
    from concourse.bass_utils import run_bass_kernel_spmd
    nc = _get_nc(1)
    in_maps = make_in_maps(Q, K, V, Wq, bq, Wk, bk, Wv, bv, Wo, bo)
    res = run_bass_kernel_spmd(nc, in_maps, core_ids=list(range(8)))
    return combine(res.results, bo)

